# revision 33
# baseline (speedup 1.0000x reference)
"""Trainium2 Bass kernel for nn_Basic_Aggregator (gnn_message_passing).

Math: out[b, i, :] = sum_j node_j[b, j, :]  (sum over node axis, broadcast
back to every row).  edge_ij is unused by the computation.

Sharding: data-parallel over batch B=16 across 8 cores (2 batches/core).
Each core reads its [2, 20000, 64] slab, reduces each batch to a [64]
vector, broadcasts it back to [20000, 64] and writes it out.  No
cross-core communication.

Pipeline per core: SWDGE loads cast f32->bf16 in the DMA engines; PE
matmuls with an all-ones lhsT both sum over the partition axis and
broadcast into a PSUM [80, 512] of row-group partial sums; a tiny DVE
reduce + doubling copies build the [80, 3200] broadcast tile; HWDGE
stores write each partition's 250 output rows as 5 repeats of that
tile.  All partition counts are divisible by 16 so every DMA spreads
over all 16 SDMA engines, and Bass(num_swdge_queues=4) selects the
full 16-engine ring-bundle table.
"""

import numpy as np

B, SIZE, D = 16, 20000, 64
N_CORES = 8
B_LOCAL = B // N_CORES  # 2
# Partition counts must be divisible by 16: the HWDGE splits one DMA
# across SDMA engines by evenly dividing the partition count with the
# largest divisor <= 16.  125 partitions -> only 5 engines (27 GB/s
# each); 128/80/32 partitions -> all 16 engines.

# Input layout: [128, 156 rows] + [32, 1 row] remainder = 20000 rows.
PIN = 128
NIN = 156               # rows per partition in the main input tile
WIN = NIN * D           # 9984 f32 per partition
PR = 32                 # remainder: rows 19968..19999, one per partition

# Output layout: [80, 250 rows]; store repeats a [50 rows] pattern 5x.
# WIDE_W must be a multiple of D (position j of a partition's slab holds
# sums[j mod 64]; a repeated block only reproduces that when the block
# is a whole number of rows) and LARGE: store descriptors below ~8 KB
# pay the HBM write latency per descriptor (2560 B descs ran at ~8 GB/s
# per engine vs 27 GB/s line rate at 12800 B).
P = 80
NG = 250
W = NG * D              # 16000 f32 per partition
WREP = 5
WIDE_W = W // WREP      # 3200 f32 (50 rows) per partition in the bcast tile

_STATE = {}

# Results of the most recent device run (for test harness introspection).
LAST_RESULT = None


def _patch_drain_split():
    """The walrus build in this container accepts at most one sync-wait
    command per instruction; Tile's kernel-tail drain collects one wait per
    dangling proc (6 here) onto a single Drain.  Split it into a chain of
    single-wait drains on the same engine — identical semantics."""
    from concourse import tile
    import concourse.mybir as mybir
    from concourse.vector_clock import ScopedClock

    if getattr(tile.TileContext, "_ant_drain_split", False):
        return

    def _drain_and_barrier(self, tick_clock, wait_clock):
        drain_inst = self.nc.sync.drain()
        wait_clock.add_sem_waits(
            drain_inst.ins, ScopedClock({None: tick_clock.global_clock})
        )
        si = drain_inst.ins.sync_info
        if si is not None and si.on_wait and len(si.on_wait) > 1:
            waits = list(si.on_wait)
            upds = list(si.on_update or [])
            drain_inst.ins.sync_info = mybir.SyncInfo(
                on_wait=[waits[0]], on_update=[]
            )
            for i, w in enumerate(waits[1:]):
                extra = self.nc.sync.drain()
                extra.ins.sync_info = mybir.SyncInfo(
                    on_wait=[w],
                    on_update=upds if i == len(waits) - 2 else [],
                )

        self.nc.all_engine_barrier()
        assert self.sems is not None
        popped = self.nc._tile_sem_poison_stack.pop()
        assert popped is self._sem_poison
        self.nc.clear_and_free_semaphores(list(self.sems.allocated().values()))
        self.nc.all_engine_barrier()

    tile.TileContext._drain_and_barrier = _drain_and_barrier
    tile.TileContext._ant_drain_split = True


def _build_nc():
    import concourse.bass as bass
    import concourse.mybir as mybir
    from concourse import tile

    _patch_drain_split()

    f32 = mybir.dt.float32
    # num_swdge_queues=4 allocates qGpSimdDynamic0-3 alongside the HWDGE
    # queues; with that many queues the runtime's ring-bundle table binds
    # ALL 16 SDMA engines to each dynamic queue (vs only 5 with the
    # default single-SWDGE-queue table) — ~2.4x the DMA bandwidth.
    nc = bass.Bass(num_swdge_queues=4)
    x = nc.declare_dram_parameter("x", [B_LOCAL, SIZE, D], f32, isOutput=False)
    y = nc.declare_dram_parameter("y", [B_LOCAL, SIZE, D], f32, isOutput=True)

    # Main-tile load chunks (f32 per partition); chunk boundaries at
    # multiples of 512 so PE matmuls consume each chunk independently.
    # b0 is finely chunked so PE starts early (under the load shadow);
    # b1 uses fewer, larger DMAs — each SWDGE dma_start costs ~0.7 us of
    # Q7 descriptor-emission bubble, and b1's matmul tail is short
    # anyway.  Loads ride the SWDGE (DMASW) sem lanes and stores the
    # HWDGE (DMAHW) lanes; each group must stay <= 8 DMAs or a
    # lane-reuse wait gets added and this walrus build rejects
    # instructions with two sync-waits.
    CHUNKS_BY_B = [[4096, 4096, 1792], [8192, 1280, 512]]
    NSLICE = 512            # matmul rhs slice width (8 rows x 64)
    N_FILLER = 24           # PE warm-keeper matmuls between batches

    bf16 = mybir.dt.bfloat16

    with tile.TileContext(nc) as tc:
        with (
            tc.tile_pool(name="io", bufs=1) as io,
            tc.tile_pool(name="small", bufs=1) as small,
            tc.tile_pool(name="psum", bufs=2, space="PSUM") as psum,
        ):
            # all-ones [PIN, P]: matmul with ones.T both sums over the
            # partition axis and broadcasts the result to P partitions:
            # (ones.T @ rhs)[p, j] = sum_k rhs[k, j] for every p.
            # bf16 operands: the loads cast f32->bf16 in the DMA engines
            # (SWDGE path), which makes the PE matmuls 4x faster than
            # fp32 (0.21 us vs 0.85 us per 512-wide slice).  The bf16
            # rounding puts ~4e-3 relative error on the sums — well
            # inside the 2e-2 gate.
            ones_t = small.tile([PIN, P], bf16, tag="ones_t")
            nc.vector.memset(ones_t[:], 1.0)
            # Pre-warm PE: the HAM takes ~4-5 us of continuous activity
            # before matmuls run at full rate, so burn small matmuls from
            # right after the memset until the first chunk arrives.  The
            # first one also absorbs the DVE-memset wait so the first
            # real matmul needs only its chunk-DMA wait (this walrus
            # build allows one sync-wait per instruction).
            warm = psum.tile([P, P], f32, tag="warm")
            for _ in range(30):
                nc.tensor.matmul(warm[:], ones_t[:], ones_t[:],
                                 start=True, stop=True)

            # Phase 1: all loads up front, chunked, casting to bf16 in
            # the DMA (gpsimd = SWDGE, the only cast-capable DGE).  Both
            # batches' tails (rows 19968..19999) come in one [32, 2, 64]
            # DMA so the load count stays within the 8 DMASW lanes; it
            # is issued AFTER b0's chunks — its 64 tiny descriptors cost
            # ~2 us of Q7 emission and would otherwise delay the first
            # big chunk (the rem sums are only needed near the end of
            # each batch's matmul chain).
            chunks = {}
            rt = small.tile([PR, B_LOCAL, D], bf16, tag="rem")
            for b in range(B_LOCAL):
                # rows 0..19967 as [128, 156*64]
                xmain = x[b][0:PIN * NIN].rearrange("(p n) d -> p (n d)", p=PIN)
                o = 0
                for c, cw in enumerate(CHUNKS_BY_B[b]):
                    t = io.tile([PIN, cw], bf16, tag=f"in{b}_{c}")
                    nc.gpsimd.dma_start(out=t[:], in_=xmain[:, o:o + cw])
                    chunks[b, c] = t
                    o += cw
                if b == 0:
                    xrem = x[:, PIN * NIN:SIZE].rearrange("b p d -> p b d")
                    nc.gpsimd.dma_start(out=rt[:], in_=xrem)

            # Phase 2: PE-accumulate row-group sums into PSUM [P, 512],
            # tiny DVE reduce + widen, store.  Stores split across the
            # two HWDGE queues (ACT, SP) so the second batch's store
            # drains concurrently with the first's.
            store_engines = [nc.scalar, nc.sync]
            warm_big = psum.tile([P, NSLICE], f32, tag="warm_big")
            # Gate ACT (store b0's queue) behind ALL of b1's loads: if
            # store b0 starts while b1 is still loading, the engines
            # round-robin between the two queues and b1's loads stretch
            # by ~15 us, delaying the critical-path b1 store.  One chained
            # gate per b1 chunk (the scheduler may complete them in any
            # order, and each instruction may carry only one sync-wait).
            for c in range(len(CHUNKS_BY_B[1])):
                gate = small.tile([1, 1], bf16, tag=f"gate{c}")
                nc.scalar.copy(gate[:], chunks[1, c][:1, :1])
            for b in range(B_LOCAL):
                acc = psum.tile([P, NSLICE], f32, tag="acc")
                n_mm = sum((cw + NSLICE - 1) // NSLICE for cw in CHUNKS_BY_B[b])
                mi = 0
                for c, cw in enumerate(CHUNKS_BY_B[b]):
                    t = chunks[b, c]
                    for o in range(0, cw, NSLICE):
                        n = min(NSLICE, cw - o)
                        mi += 1
                        if mi == n_mm:
                            # remainder rows 19968..19999 fold into
                            # group 0's slots (free offsets 0..63) just
                            # before the stop matmul (the rem tile loads
                            # after b0's chunks).
                            nc.tensor.matmul(acc[:, 0:D], ones_t[:PR, :],
                                             rt[:, b, :],
                                             start=False, stop=False)
                        # first matmul (full 512 wide) resets the bank
                        nc.tensor.matmul(acc[:, 0:n], ones_t[:],
                                         t[:, o:o + n],
                                         start=(mi == 1), stop=(mi == n_mm))

                # acc[p, (g, d)] holds 8 partial row-group sums; fold the
                # g axis (512 elems) straight into the bcast tile, then
                # widen by doubling copies (wide[j] = sums[j mod 64]).
                wide = io.tile([P, WIDE_W], f32, tag=f"wide{b}")
                gview = acc[:].rearrange("p (g d) -> p d g", d=D)
                nc.vector.reduce_sum(wide[:, 0:D], gview,
                                     axis=mybir.AxisListType.X)
                w = D
                while w < WIDE_W:
                    c = min(w, WIDE_W - w)
                    nc.vector.tensor_copy(wide[:, w:w + c], wide[:, 0:c])
                    w += c

                # store with a free-axis repeat: each partition's 16000
                # output f32 are 20 repeats of the 800-f32 pattern.
                yb = y[b].rearrange("(p a) d -> p (a d)", p=P)
                yb = yb.rearrange("p (r w) -> p r w", r=WREP)
                src = wide[:].unsqueeze(1).broadcast_to([P, WREP, WIDE_W])
                store_engines[b].dma_start(out=yb, in_=src)

                if b == 0:
                    # PE cools (HAM) during the idle window between the
                    # batches and then runs the next batch's matmuls at
                    # half rate; keep it warm with throwaway matmuls.
                    for _ in range(N_FILLER):
                        nc.tensor.matmul(warm_big[:], ones_t[:],
                                         chunks[0, 0][:, 0:NSLICE],
                                         start=True, stop=True)

    return nc


def _get_nc():
    if "nc" not in _STATE:
        _STATE["nc"] = _build_nc()
    return _STATE["nc"]


def kernel(node_j, edge_ij=None):
    global LAST_RESULT
    from concourse.bass_utils import run_bass_kernel_spmd

    node_j = np.ascontiguousarray(np.asarray(node_j), dtype=np.float32)
    assert node_j.shape == (B, SIZE, D), node_j.shape

    nc = _get_nc()
    in_maps = [
        {"x": node_j[i * B_LOCAL:(i + 1) * B_LOCAL]} for i in range(N_CORES)
    ]
    res = run_bass_kernel_spmd(nc, in_maps, core_ids=list(range(N_CORES)))
    LAST_RESULT = res
    out = np.concatenate([r["y"] for r in res.results], axis=0)
    return out



# revision 36
# speedup vs baseline: 1.0003x; 1.0003x over previous
"""Trainium2 Bass kernel for nn_Basic_Aggregator (gnn_message_passing).

Math: out[b, i, :] = sum_j node_j[b, j, :]  (sum over node axis, broadcast
back to every row).  edge_ij is unused by the computation.

Sharding: data-parallel over batch B=16 across 8 cores (2 batches/core).
Each core reads its [2, 20000, 64] slab, reduces each batch to a [64]
vector, broadcasts it back to [20000, 64] and writes it out.  No
cross-core communication.

Pipeline per core: SWDGE loads cast f32->bf16 in the DMA engines; PE
matmuls with an all-ones lhsT both sum over the partition axis and
broadcast into a PSUM [80, 512] of row-group partial sums; a tiny DVE
reduce + doubling copies build the [80, 3200] broadcast tile; HWDGE
stores write each partition's 250 output rows as 5 repeats of that
tile.  All partition counts are divisible by 16 so every DMA spreads
over all 16 SDMA engines, and Bass(num_swdge_queues=4) selects the
full 16-engine ring-bundle table.
"""

import numpy as np

B, SIZE, D = 16, 20000, 64
N_CORES = 8
B_LOCAL = B // N_CORES  # 2
# Partition counts must be divisible by 16: the HWDGE splits one DMA
# across SDMA engines by evenly dividing the partition count with the
# largest divisor <= 16.  125 partitions -> only 5 engines (27 GB/s
# each); 128/80/32 partitions -> all 16 engines.

# Input layout: [128, 156 rows] + [32, 1 row] remainder = 20000 rows.
PIN = 128
NIN = 156               # rows per partition in the main input tile
WIN = NIN * D           # 9984 f32 per partition
PR = 32                 # remainder: rows 19968..19999, one per partition

# Output layout: [80, 250 rows]; store repeats a [50 rows] pattern 5x.
# WIDE_W must be a multiple of D (position j of a partition's slab holds
# sums[j mod 64]; a repeated block only reproduces that when the block
# is a whole number of rows) and LARGE: store descriptors below ~8 KB
# pay the HBM write latency per descriptor (2560 B descs ran at ~8 GB/s
# per engine vs 27 GB/s line rate at 12800 B).
P = 80
NG = 250
W = NG * D              # 16000 f32 per partition
WREP = 5
WIDE_W = W // WREP      # 3200 f32 (50 rows) per partition in the bcast tile

_STATE = {}

# Results of the most recent device run (for test harness introspection).
LAST_RESULT = None


def _patch_drain_split():
    """The walrus build in this container accepts at most one sync-wait
    command per instruction; Tile's kernel-tail drain collects one wait per
    dangling proc (6 here) onto a single Drain.  Split it into a chain of
    single-wait drains on the same engine — identical semantics."""
    from concourse import tile
    import concourse.mybir as mybir
    from concourse.vector_clock import ScopedClock

    if getattr(tile.TileContext, "_ant_drain_split", False):
        return

    def _drain_and_barrier(self, tick_clock, wait_clock):
        drain_inst = self.nc.sync.drain()
        wait_clock.add_sem_waits(
            drain_inst.ins, ScopedClock({None: tick_clock.global_clock})
        )
        si = drain_inst.ins.sync_info
        if si is not None and si.on_wait and len(si.on_wait) > 1:
            waits = list(si.on_wait)
            upds = list(si.on_update or [])
            drain_inst.ins.sync_info = mybir.SyncInfo(
                on_wait=[waits[0]], on_update=[]
            )
            for i, w in enumerate(waits[1:]):
                extra = self.nc.sync.drain()
                extra.ins.sync_info = mybir.SyncInfo(
                    on_wait=[w],
                    on_update=upds if i == len(waits) - 2 else [],
                )

        self.nc.all_engine_barrier()
        assert self.sems is not None
        popped = self.nc._tile_sem_poison_stack.pop()
        assert popped is self._sem_poison
        self.nc.clear_and_free_semaphores(list(self.sems.allocated().values()))
        self.nc.all_engine_barrier()

    tile.TileContext._drain_and_barrier = _drain_and_barrier
    tile.TileContext._ant_drain_split = True


def _build_nc():
    import concourse.bass as bass
    import concourse.mybir as mybir
    from concourse import tile

    _patch_drain_split()

    f32 = mybir.dt.float32
    # num_swdge_queues=4 allocates qGpSimdDynamic0-3 alongside the HWDGE
    # queues; with that many queues the runtime's ring-bundle table binds
    # ALL 16 SDMA engines to each dynamic queue (vs only 5 with the
    # default single-SWDGE-queue table) — ~2.4x the DMA bandwidth.
    nc = bass.Bass(num_swdge_queues=4)
    x = nc.declare_dram_parameter("x", [B_LOCAL, SIZE, D], f32, isOutput=False)
    y = nc.declare_dram_parameter("y", [B_LOCAL, SIZE, D], f32, isOutput=True)

    # Main-tile load chunks (f32 per partition); chunk boundaries at
    # multiples of 512 so PE matmuls consume each chunk independently.
    # b0 is finely chunked so PE starts early (under the load shadow);
    # b1 uses fewer, larger DMAs — each SWDGE dma_start costs ~0.7 us of
    # Q7 descriptor-emission bubble, and b1's matmul tail is short
    # anyway.  Loads ride the SWDGE (DMASW) sem lanes and stores the
    # HWDGE (DMAHW) lanes; each group must stay <= 8 DMAs or a
    # lane-reuse wait gets added and this walrus build rejects
    # instructions with two sync-waits.
    CHUNKS_BY_B = [[4096, 4096, 1792], [8192, 1792]]
    NSLICE = 512            # matmul rhs slice width (8 rows x 64)
    N_FILLER = 24           # PE warm-keeper matmuls between batches

    bf16 = mybir.dt.bfloat16

    with tile.TileContext(nc) as tc:
        with (
            tc.tile_pool(name="io", bufs=1) as io,
            tc.tile_pool(name="small", bufs=1) as small,
            tc.tile_pool(name="psum", bufs=2, space="PSUM") as psum,
        ):
            # all-ones [PIN, P]: matmul with ones.T both sums over the
            # partition axis and broadcasts the result to P partitions:
            # (ones.T @ rhs)[p, j] = sum_k rhs[k, j] for every p.
            # bf16 operands: the loads cast f32->bf16 in the DMA engines
            # (SWDGE path), which makes the PE matmuls 4x faster than
            # fp32 (0.21 us vs 0.85 us per 512-wide slice).  The bf16
            # rounding puts ~4e-3 relative error on the sums — well
            # inside the 2e-2 gate.
            ones_t = small.tile([PIN, P], bf16, tag="ones_t")
            nc.vector.memset(ones_t[:], 1.0)
            # Pre-warm PE: the HAM takes ~4-5 us of continuous activity
            # before matmuls run at full rate, so burn small matmuls from
            # right after the memset until the first chunk arrives.  The
            # first one also absorbs the DVE-memset wait so the first
            # real matmul needs only its chunk-DMA wait (this walrus
            # build allows one sync-wait per instruction).
            warm = psum.tile([P, P], f32, tag="warm")
            for _ in range(30):
                nc.tensor.matmul(warm[:], ones_t[:], ones_t[:],
                                 start=True, stop=True)

            # Phase 1: all loads up front, chunked, casting to bf16 in
            # the DMA (gpsimd = SWDGE, the only cast-capable DGE).  Both
            # batches' tails (rows 19968..19999) come in one [32, 2, 64]
            # DMA so the load count stays within the 8 DMASW lanes.
            rt = small.tile([PR, B_LOCAL, D], bf16, tag="rem")
            xrem = x[:, PIN * NIN:SIZE].rearrange("b p d -> p b d")
            nc.gpsimd.dma_start(out=rt[:], in_=xrem)
            chunks = {}
            for b in range(B_LOCAL):
                # rows 0..19967 as [128, 156*64]
                xmain = x[b][0:PIN * NIN].rearrange("(p n) d -> p (n d)", p=PIN)
                o = 0
                for c, cw in enumerate(CHUNKS_BY_B[b]):
                    t = io.tile([PIN, cw], bf16, tag=f"in{b}_{c}")
                    nc.gpsimd.dma_start(out=t[:], in_=xmain[:, o:o + cw])
                    chunks[b, c] = t
                    o += cw

            # Phase 2: PE-accumulate row-group sums into PSUM [P, 512],
            # tiny DVE reduce + widen, store.  Stores split across the
            # two HWDGE queues (ACT, SP) so the second batch's store
            # drains concurrently with the first's.
            store_engines = [nc.scalar, nc.sync]
            warm_big = psum.tile([P, NSLICE], f32, tag="warm_big")
            # Gate ACT (store b0's queue) behind ALL of b1's loads: if
            # store b0 starts while b1 is still loading, the engines
            # round-robin between the two queues and b1's loads stretch
            # by ~15 us, delaying the critical-path b1 store.  One chained
            # gate per b1 chunk (the scheduler may complete them in any
            # order, and each instruction may carry only one sync-wait).
            for c in range(len(CHUNKS_BY_B[1])):
                gate = small.tile([1, 1], bf16, tag=f"gate{c}")
                nc.scalar.copy(gate[:], chunks[1, c][:1, :1])
            for b in range(B_LOCAL):
                acc = psum.tile([P, NSLICE], f32, tag="acc")
                n_mm = sum((cw + NSLICE - 1) // NSLICE for cw in CHUNKS_BY_B[b])
                mi = 0
                for c, cw in enumerate(CHUNKS_BY_B[b]):
                    t = chunks[b, c]
                    for o in range(0, cw, NSLICE):
                        n = min(NSLICE, cw - o)
                        mi += 1
                        # first matmul (full 512 wide) resets the bank
                        nc.tensor.matmul(acc[:, 0:n], ones_t[:],
                                         t[:, o:o + n],
                                         start=(mi == 1), stop=(mi == n_mm))
                        if mi == 1:
                            # remainder rows 19968..19999 fold into
                            # group 0's slots (free offsets 0..63).
                            nc.tensor.matmul(acc[:, 0:D], ones_t[:PR, :],
                                             rt[:, b, :],
                                             start=False, stop=False)

                # acc[p, (g, d)] holds 8 partial row-group sums; fold the
                # g axis (512 elems) straight into the bcast tile, then
                # widen by doubling copies (wide[j] = sums[j mod 64]).
                wide = io.tile([P, WIDE_W], f32, tag=f"wide{b}")
                gview = acc[:].rearrange("p (g d) -> p d g", d=D)
                nc.vector.reduce_sum(wide[:, 0:D], gview,
                                     axis=mybir.AxisListType.X)
                w = D
                while w < WIDE_W:
                    c = min(w, WIDE_W - w)
                    nc.vector.tensor_copy(wide[:, w:w + c], wide[:, 0:c])
                    w += c

                # store with a free-axis repeat: each partition's 16000
                # output f32 are 20 repeats of the 800-f32 pattern.
                yb = y[b].rearrange("(p a) d -> p (a d)", p=P)
                yb = yb.rearrange("p (r w) -> p r w", r=WREP)
                src = wide[:].unsqueeze(1).broadcast_to([P, WREP, WIDE_W])
                store_engines[b].dma_start(out=yb, in_=src)

                if b == 0:
                    # PE cools (HAM) during the idle window between the
                    # batches and then runs the next batch's matmuls at
                    # half rate; keep it warm with throwaway matmuls.
                    for _ in range(N_FILLER):
                        nc.tensor.matmul(warm_big[:], ones_t[:],
                                         chunks[0, 0][:, 0:NSLICE],
                                         start=True, stop=True)

    return nc


def _get_nc():
    if "nc" not in _STATE:
        _STATE["nc"] = _build_nc()
    return _STATE["nc"]


def kernel(node_j, edge_ij=None):
    global LAST_RESULT
    from concourse.bass_utils import run_bass_kernel_spmd

    node_j = np.ascontiguousarray(np.asarray(node_j), dtype=np.float32)
    assert node_j.shape == (B, SIZE, D), node_j.shape

    nc = _get_nc()
    in_maps = [
        {"x": node_j[i * B_LOCAL:(i + 1) * B_LOCAL]} for i in range(N_CORES)
    ]
    res = run_bass_kernel_spmd(nc, in_maps, core_ids=list(range(N_CORES)))
    LAST_RESULT = res
    out = np.concatenate([r["y"] for r in res.results], axis=0)
    return out

